# revision 1
# baseline (speedup 1.0000x reference)
"""LoRA-injected 3x3 conv (MoE-routed adapters), Trainium2 Bass kernel.

Strategy:
 - Host: merge each sample's LoRA adapter into the base conv weight
   (W_eff = conv_w + scale*active * up @ down  -- exact low-rank merge),
   pre-transpose weights to [ci, tap, co] (matmul lhsT layout), zero-pad x
   spatially, shard batch across 8 cores (2 samples each).
 - Device: 3x3 conv as PE matmuls (fp32r: ~1 column/cycle). ci=320 splits
   into partition chunks 128+128+64. The 64-wide tail chunk is packed to
   full K=128 by storing shifted copies of the image in the upper 64
   partitions (shift +2 pairs taps (kh,0)+(kh,2); shift +66 pairs
   (0,1)+(1,1)), so each 512-wide output chunk needs 23 accumulating
   matmuls instead of 27. Weights are the stationary operand and are
   reused across 4 PSUM banks (k-outer, spatial-inner) to amortize
   LDWEIGHTS. Bias-add fused into the PSUM->SBUF copy on ScalarE.
"""

import sys

for _p in ("/opt/trn_rl_repo",):
    if _p not in sys.path:
        sys.path.insert(0, _p)

import numpy as np

B, CIN, COUT, H, W = 16, 320, 320, 64, 64
R, NUM_LORAS, LORA_STRIDE, SCALE = 4, 50, 4, 1.0
NCORES = 8
BLOC = B // NCORES          # samples per core
HP, WP = H + 2, W + 2       # padded spatial
SP = HP * WP                # padded flat spatial per channel
HWFLAT = H * W
HHALF = H // 2              # rows per half-image
HALF_IN = (HHALF + 2) * WP  # 2244 padded elements per half
HALF_BASE = HHALF * WP      # 2112 flat offset of second half
NSL = 4                     # 512-wide spatial chunks per half
SPC = 512
FULL_CHUNKS = [(0, 128), (128, 128)]
CO_CHUNKS = [(0, 128), (128, 128), (256, 64)]
# tap pairs packed into the upper 64 partitions of the ci tail chunk:
# (tap_lo, tap_hi, which shifted tile, kh, kw of tap_lo)
TAIL_TAPS = [
    (0, 2, "a", 0, 0),   # (0,0)+(0,2): shift +2
    (3, 5, "a", 1, 0),   # (1,0)+(1,2)
    (6, 8, "a", 2, 0),   # (2,0)+(2,2)
    (1, 4, "b", 0, 1),   # (0,1)+(1,1): shift +66
    (7, None, "a", 2, 1),  # (2,1) alone, K=64
]

_NC_CACHE = {}


def _build_nc():
    import concourse.bacc as bacc
    import concourse.bass as bass
    import concourse.mybir as mybir
    from concourse import tile

    f32 = mybir.dt.float32
    f32r = mybir.dt.float32r

    nc = bacc.Bacc(None, target_bir_lowering=False)

    xp_d = nc.dram_tensor("xp", [BLOC, CIN, SP], f32r, kind="ExternalInput")
    wt_d = nc.dram_tensor("wt", [BLOC, 256, 9 * COUT], f32r, kind="ExternalInput")
    wp_d = nc.dram_tensor("wp", [BLOC, 128, 5 * COUT], f32r, kind="ExternalInput")
    bias_d = nc.dram_tensor("bias", [128, 3], f32, kind="ExternalInput")
    y_d = nc.dram_tensor("y", [BLOC, COUT, HWFLAT], f32, kind="ExternalOutput")

    with tile.TileContext(nc) as tc:
        with (
            tc.tile_pool(name="io", bufs=2) as io_pool,
            tc.tile_pool(name="const", bufs=1) as cpool,
            tc.tile_pool(name="ostage", bufs=4) as opool,
            tc.tile_pool(name="acc", bufs=8, space=bass.MemorySpace.PSUM) as pspool,
        ):
            bias_t = cpool.tile([128, 3], f32, tag="bias")

            for b in range(BLOC):
                # initial loads spread across idle engine queues so the
                # first matmul group (needs x0+w0 only) gates on ~1.5MB,
                # not the whole sample's working set on one queue
                w01 = []
                for kc, (c0, _) in enumerate(FULL_CHUNKS):
                    wt = io_pool.tile([128, 9 * COUT], f32r, tag=f"w{kc}")
                    eng = nc.gpsimd if kc == 0 else nc.scalar
                    eng.dma_start(out=wt[:], in_=wt_d[b, c0 : c0 + 128, :])
                    w01.append(wt)
                wp = io_pool.tile([128, 5 * COUT], f32r, tag="wp")
                nc.gpsimd.dma_start(out=wp[:], in_=wp_d[b])

                for half in range(2):
                    base = half * HALF_BASE
                    xts = []
                    for kc, (c0, _) in enumerate(FULL_CHUNKS):
                        xt = io_pool.tile([128, HALF_IN], f32r, tag=f"x{kc}")
                        nc.sync.dma_start(
                            out=xt[:], in_=xp_d[b, c0 : c0 + 128, base : base + HALF_IN]
                        )
                        xts.append(xt)
                    # ci tail chunk (64 rows) + shifted copies in partitions 64..127
                    xa = io_pool.tile([128, HALF_IN], f32r, tag="xa")
                    nc.gpsimd.dma_start(out=xa[:64], in_=xp_d[b, 256:320, base : base + HALF_IN])
                    nc.gpsimd.dma_start(
                        out=xa[64:128, 0 : HALF_IN - 2],
                        in_=xp_d[b, 256:320, base + 2 : base + HALF_IN],
                    )
                    xb = io_pool.tile([128, HALF_IN], f32r, tag="xb")
                    nc.sync.dma_start(out=xb[:64], in_=xp_d[b, 256:320, base : base + HALF_IN])
                    nc.sync.dma_start(
                        out=xb[64:128, 0 : HALF_IN - WP],
                        in_=xp_d[b, 256:320, base + WP : base + HALF_IN],
                    )
                    if b == 0 and half == 0:
                        nc.scalar.dma_start(out=bias_t[:], in_=bias_d[:])

                    xvs = [t[:].rearrange("p (h w) -> p h w", w=WP) for t in xts]
                    xav = xa[:].rearrange("p (h w) -> p h w", w=WP)
                    xbv = xb[:].rearrange("p (h w) -> p h w", w=WP)
                    wvs = [t[:].rearrange("p (t c) -> p t c", c=COUT) for t in w01]
                    wpv = wp[:].rearrange("p (q c) -> p q c", c=COUT)

                    for cc, (o0, osz) in enumerate(CO_CHUNKS):
                        # (lhsT, rhs-builder, K) per accumulation step
                        ents = []
                        for kc in range(2):
                            for kh in range(3):
                                for kw in range(3):
                                    ents.append(
                                        (
                                            wvs[kc][:, kh * 3 + kw, o0 : o0 + osz],
                                            (xvs[kc], kh, kw),
                                            128,
                                        )
                                    )
                        for q, (tlo, thi, which, kh, kw) in enumerate(TAIL_TAPS):
                            src = xav if which == "a" else xbv
                            ksz = 128 if thi is not None else 64
                            ents.append(
                                (wpv[:ksz, q, o0 : o0 + osz], (src, kh, kw), ksz)
                            )

                        pss = [
                            pspool.tile([128, SPC], f32, tag="ps", name=f"ps{sl}")
                            for sl in range(NSL)
                        ]
                        last = len(ents) - 1
                        for kj, (lhsT, (src, kh, kw), ksz) in enumerate(ents):
                            for sl in range(NSL):
                                rhs = src[:ksz, 8 * sl + kh : 8 * sl + kh + 8, kw : kw + W]
                                nc.tensor.matmul(
                                    pss[sl][:osz],
                                    lhsT,
                                    rhs,
                                    start=(kj == 0),
                                    stop=(kj == last),
                                )
                        for sl in range(NSL):
                            ob = opool.tile([128, SPC], f32, tag="ob")
                            nc.scalar.activation(
                                ob[:osz],
                                pss[sl][:osz],
                                mybir.ActivationFunctionType.Identity,
                                bias=bias_t[:osz, cc : cc + 1],
                            )
                            o_off = half * (HWFLAT // 2) + sl * SPC
                            nc.sync.dma_start(
                                out=y_d[b, o0 : o0 + osz, o_off : o_off + SPC],
                                in_=ob[:osz],
                            )

    nc.compile()
    return nc


def _get_nc():
    if "nc" not in _NC_CACHE:
        _NC_CACHE["nc"] = _build_nc()
    return _NC_CACHE["nc"]


def _prep_inputs(x, conv_w, conv_b, down_w, up_w, lora_id):
    x = np.asarray(x, dtype=np.float32)
    conv_w = np.asarray(conv_w, dtype=np.float32)
    conv_b = np.asarray(conv_b, dtype=np.float32)
    down_w = np.asarray(down_w, dtype=np.float32)
    up_w = np.asarray(up_w, dtype=np.float32)
    idx = np.asarray(lora_id).astype(np.int64) // LORA_STRIDE
    active = (idx >= 0).astype(np.float32)
    safe = np.clip(idx, 0, NUM_LORAS - 1)

    # Exact LoRA merge: W_lora[b,o,i,kh,kw] = sum_r up[o,r] down[r,i,kh,kw]
    lora = np.matmul(up_w[safe], down_w[safe].reshape(B, R, -1))
    lora = lora.reshape(B, COUT, CIN, 3, 3)
    weff = conv_w[None] + (SCALE * active)[:, None, None, None, None] * lora
    # lhsT layout [b, ci, tap, co]
    wt9 = np.ascontiguousarray(weff.transpose(0, 2, 3, 4, 1)).reshape(B, CIN, 9, COUT)
    wt_main = wt9[:, :256].reshape(B, 256, 9 * COUT)
    # paired tail-chunk weights: [b, 128, 5, co]
    wp_all = np.zeros((B, 128, 5, COUT), dtype=np.float32)
    for q, (tlo, thi, _, _, _) in enumerate(TAIL_TAPS):
        wp_all[:, 0:64, q] = wt9[:, 256:320, tlo]
        if thi is not None:
            wp_all[:, 64:128, q] = wt9[:, 256:320, thi]
    wp_all = wp_all.reshape(B, 128, 5 * COUT)

    xp = np.pad(x, ((0, 0), (0, 0), (1, 1), (1, 1))).reshape(B, CIN, SP)
    bias2 = np.zeros((128, 3), dtype=np.float32)
    for cc, (o0, osz) in enumerate(CO_CHUNKS):
        bias2[:osz, cc] = conv_b[o0 : o0 + osz]

    in_maps = [
        {
            "xp": np.ascontiguousarray(xp[c * BLOC : (c + 1) * BLOC]),
            "wt": np.ascontiguousarray(wt_main[c * BLOC : (c + 1) * BLOC]),
            "wp": np.ascontiguousarray(wp_all[c * BLOC : (c + 1) * BLOC]),
            "bias": bias2,
        }
        for c in range(NCORES)
    ]
    return in_maps


def run_device(in_maps, trace=False, tmpdir=None):
    from concourse.bass_utils import run_bass_kernel_spmd

    nc = _get_nc()
    return run_bass_kernel_spmd(
        nc, in_maps, list(range(NCORES)), trace=trace, tmpdir=tmpdir
    )


def kernel(x, conv_w, conv_b, down_w, up_w, lora_id):
    in_maps = _prep_inputs(x, conv_w, conv_b, down_w, up_w, lora_id)
    out = run_device(in_maps)
    y = np.concatenate([out.results[c]["y"] for c in range(NCORES)], axis=0)
    return np.ascontiguousarray(y.reshape(B, COUT, H, W).astype(np.float32))

